# revision 1
# baseline (speedup 1.0000x reference)
"""Masked self-attention (softmax over axis=1) Bass kernel for TRN2, 8 cores.

Reference semantics (per batch b):
    attn[l, m] = <a_l, a_m> * temperature            [L, L]
    attn = where(mask[l, m], attn, -1e7)
    P = softmax(attn, axis=l)                        (softmax over dim 0)
    out[m, :] = sum_l P[l, m] * a[l, :]              [L, H]

Equivalently out = softmax_cols(masked scores)^T @ a. We compute, per core
(4 batches each, pure data parallel across 8 cores, no collectives):

    a -> SBUF (f32), DVE cast -> bf16, bounce to DRAM, one DMA-xbar
        transpose -> AT [768, 1024] bf16 (cheapest transpose path: the
        xbar needs a 2-byte dtype and an SBUF destination)
    S[l, m] row-tile [128, 1024] = sum_d AT[d, l-tile]^T @ AT[d, :]  (PE)
        - paired chunk matmuls share the loaded weights (the 2nd
          InstMatmult gets .ldweights = False)
        - S is symmetric pre-mask: the lower-left chunk of rows 4-7 is a
          PE transpose of parked upper-right chunks of rows 0-3
          (256 vs 3072 PE cycles per 128x512 chunk)
    S' = mask_u8 * (BIG/temp) + S    (DVE scalar_tensor_tensor, one pass)
    E = exp(temp * S' - BIG) -> bf16 (ACT, one pass; masked entries
        become ~e^-50 ~ 0, kept entries are exactly exp(temp*S))
    [feat | denom] = E^T @ [a | 1]   (PE; ones column makes the softmax
        denominator fall out of the same matmul)
    out = feat * (1/denom)           (DVE reciprocal + ACT scale-copy)

No max-subtraction needed: scores*temp ~ N(0,1), diagonal ~ +28, exp stays
well inside fp32 range; accumulation is fp32 in PSUM. DMA ring discipline:
staging on the scalar HWDGE ring (plus sync ring for batch 0), bounces on
sync, xbar transposes exclusively on sync (mode switches serialize a
ring), outputs on gpsimd SWDGE.
"""

import sys

import numpy as np

sys.path.insert(0, "/opt/trn_rl_repo")

B, L, H = 32, 1024, 768
N_CORES = 8
B_LOCAL = B // N_CORES  # 4 batches per core
LT = L // 128  # 8 l-tiles
DT = H // 128  # 6 d-tiles
BIG = 50.0
REUSE_WEIGHTS = True
import os as _os

STAGE_RING2 = _os.environ.get("K_RING2", "hybrid")  # 2nd staging ring
N_TRANSPOSE = int(_os.environ.get("K_NTRANS", "1"))  # 1 or 6 transpose DMAs
STAGE_DELAY = float(_os.environ.get("K_DELAY", "0.0"))  # ms per batch index
SYM = int(_os.environ.get("K_SYM", "1"))  # lower-left S blocks via PE transpose
OUT_RING = _os.environ.get("K_OUT", "gpsimd")  # out-DMA engine
CAST2 = _os.environ.get("K_CAST2", "vector")  # engine for 2nd half cast
BOUNCE_RING = _os.environ.get("K_BOUNCE", "sync")  # same|sync
MASK_DELAY = float(_os.environ.get("K_MASKD", "0.012"))  # ms, batch-0 mask hold

_CACHE = {}


def _build(temp: float, repeats: int = 1, bench: bool = False):
    from contextlib import ExitStack

    import concourse.bass as bass
    import concourse.mybir as mybir
    from concourse import bacc, tile

    f32 = mybir.dt.float32
    bf16 = mybir.dt.bfloat16
    u8 = mybir.dt.uint8

    nc = bacc.Bacc(
        "TRN2", target_bir_lowering=False, debug=False, num_devices=N_CORES
    )

    if bench:
        # Timing-only variant: big tensors live in Internal DRAM (content
        # irrelevant — instruction stream is identical), so per-call axon
        # transfer overhead stays tiny and the R-repeat delta is clean.
        nc.dram_tensor("bench_in", [1, 4], f32, kind="ExternalInput")
        nc.dram_tensor("out", [1, 4], f32, kind="ExternalOutput")
        a_ext = nc.dram_tensor("a", [B_LOCAL, L, H], f32).ap()
        m_ext = nc.dram_tensor("mask_a", [B_LOCAL, L, L], u8).ap()
        out_ext = nc.dram_tensor("out_int", [B_LOCAL, L, H], f32).ap()
    else:
        a_ext = nc.dram_tensor("a", [B_LOCAL, L, H], f32, kind="ExternalInput").ap()
        m_ext = nc.dram_tensor(
            "mask_a", [B_LOCAL, L, L], u8, kind="ExternalInput"
        ).ap()
        out_ext = nc.dram_tensor(
            "out", [B_LOCAL, L, H], f32, kind="ExternalOutput"
        ).ap()

    big_over_temp = BIG / temp

    with tile.TileContext(nc) as tc, ExitStack() as ctx:
        a_pool = ctx.enter_context(tc.tile_pool(name="asb", bufs=2))
        t2_pool = ctx.enter_context(tc.tile_pool(name="t2", bufs=2))
        at_pool = ctx.enter_context(tc.tile_pool(name="at", bufs=2))
        mask_pool = ctx.enter_context(tc.tile_pool(name="mask", bufs=2))
        e_pool = ctx.enter_context(tc.tile_pool(name="e", bufs=2))
        sp_pool = ctx.enter_context(tc.tile_pool(name="sp", bufs=4))
        out_pool = ctx.enter_context(tc.tile_pool(name="outp", bufs=3))
        rc_pool = ctx.enter_context(tc.tile_pool(name="rc", bufs=3))
        dram_pool = ctx.enter_context(
            tc.tile_pool(name="bounce", bufs=2, space="DRAM")
        )
        psum_s = ctx.enter_context(tc.tile_pool(name="ps_s", bufs=2, space="PSUM"))
        psum_o = ctx.enter_context(tc.tile_pool(name="ps_o", bufs=2, space="PSUM"))
        const_pool = ctx.enter_context(tc.tile_pool(name="const", bufs=1))

        neg_big = const_pool.tile([128, 1], f32)
        nc.vector.memset(neg_big[:], -BIG)
        if SYM:
            from concourse.masks import make_identity

            park_pool = ctx.enter_context(tc.tile_pool(name="park", bufs=5))
            ident = const_pool.tile([128, 128], f32)
            make_identity(nc, ident[:])

        for bi, b in enumerate(
            [b for _ in range(repeats) for b in range(B_LOCAL)]
        ):
            a_v = a_ext[b].rearrange("(i p) d -> p i d", p=128)  # [128, 8, 768]
            m_v = m_ext[b].rearrange("(i p) m -> p i m", p=128)  # [128, 8, 1024]
            o_v = out_ext[b].rearrange("(i p) d -> p i d", p=128)

            # Stage: a (f32) -> SBUF, DVE cast to bf16, bounce to DRAM bf16,
            # one xbar-transpose DMA -> AT. The sync queue carries ONLY
            # transposes (xbar-mode switches serialize a HWDGE queue).
            asb = a_pool.tile([128, LT, H], f32)
            t2 = t2_pool.tile([128, LT, H + 1], bf16)
            scratch = dram_pool.tile([L, H], bf16)
            s_v = scratch[:].rearrange("(i p) d -> p i d", p=128)
            at = at_pool.tile([128, DT, L], bf16)
            msk = mask_pool.tile([128, LT, L], u8)
            # Batch 0: split the a-load/bounce across both HWDGE rings to
            # halve the pipeline-fill latency; later batches stage on the
            # scalar ring only (their staging hides behind compute, and
            # keeping the sync ring transpose-only avoids xbar-mode churn).
            # Later batches' staging is also held back in model time so it
            # cannot cut ahead of the previous batch's critical chain.
            use2 = bi == 0 and STAGE_RING2 == "hybrid"
            dma_eng = (
                nc.scalar,
                nc.sync if (STAGE_RING2 == "sync" or use2) else nc.scalar,
            )
            with tc.tile_wait_until(bi * STAGE_DELAY, enable=bi > 0):
                cast_eng = (nc.vector, nc.gpsimd if CAST2 == "gpsimd" else nc.vector)
                for hh in range(2):
                    rsl = slice(4 * hh, 4 * (hh + 1))
                    dma_eng[hh].dma_start(out=asb[:, rsl, :], in_=a_v[:, rsl, :])
                    cast_eng[hh].tensor_copy(t2[:, rsl, 0:H], asb[:, rsl, :])
                    b_eng = nc.sync if BOUNCE_RING == "sync" else dma_eng[hh]
                    b_eng.dma_start(out=s_v[:, rsl, :], in_=t2[:, rsl, 0:H])
                if N_TRANSPOSE == 1:
                    nc.sync.dma_start(
                        out=at[:, :, :], in_=scratch[:, :], transpose=True
                    )
                elif N_TRANSPOSE == 2:
                    for hh in range(2):
                        rows = slice(512 * hh, 512 * (hh + 1))
                        nc.sync.dma_start(
                            out=at[:, :, rows],
                            in_=scratch[rows, :],
                            transpose=True,
                        )
                else:
                    for j in range(DT):
                        nc.sync.dma_start(
                            out=at[:, j, :],
                            in_=scratch[:, 128 * j : 128 * (j + 1)],
                            transpose=True,
                        )
                nc.vector.memset(t2[:, :, H : H + 1], 1.0)
                with tc.tile_wait_until(MASK_DELAY, enable=bi == 0 and MASK_DELAY > 0):
                    nc.scalar.dma_start(out=msk[:], in_=m_v)

            # E[l, m] = exp(temp*S + (mask-1)*BIG), bf16, [128, 8, 1024]
            e = e_pool.tile([128, LT, L], bf16)
            parks = {}
            for li in range(LT):
                ps = psum_s.tile([128, L], f32)  # 2 banks, chunks c=0/1
                if SYM and li >= LT // 2:
                    # Lower-left chunk = transpose of parked upper-right
                    # blocks (S is symmetric before masking).
                    for mj in range(LT // 2):
                        nc.tensor.transpose(
                            ps[:, 128 * mj : 128 * (mj + 1)],
                            parks[mj][:, 128 * (li - 4) : 128 * (li - 4) + 128],
                            ident[:],
                        )
                    for j in range(DT):
                        nc.tensor.matmul(
                            ps[:, 512:1024],
                            lhsT=at[:, j, 128 * li : 128 * (li + 1)],
                            rhs=at[:, j, 512:1024],
                            start=(j == 0),
                            stop=(j == DT - 1),
                        )
                else:
                    for j in range(DT):
                        nc.tensor.matmul(
                            ps[:, 0:512],
                            lhsT=at[:, j, 128 * li : 128 * (li + 1)],
                            rhs=at[:, j, 0:512],
                            start=(j == 0),
                            stop=(j == DT - 1),
                        )
                        mm2nd = nc.tensor.matmul(
                            ps[:, 512:1024],
                            lhsT=at[:, j, 128 * li : 128 * (li + 1)],
                            rhs=at[:, j, 512:1024],
                            start=(j == 0),
                            stop=(j == DT - 1),
                        )
                        if REUSE_WEIGHTS:
                            mm2nd.ins.ldweights = False
                    if SYM and li < LT // 2:
                        pk = park_pool.tile([128, 512], f32)
                        nc.vector.tensor_copy(pk[:], ps[:, 512:1024])
                        parks[li] = pk
                sp = sp_pool.tile([128, L], f32)
                nc.vector.scalar_tensor_tensor(
                    out=sp[:],
                    in0=msk[:, li, :],
                    scalar=big_over_temp,
                    in1=ps[:],
                    op0=mybir.AluOpType.mult,
                    op1=mybir.AluOpType.add,
                )
                nc.scalar.activation(
                    out=e[:, li, :],
                    in_=sp[:],
                    func=mybir.ActivationFunctionType.Exp,
                    bias=neg_big[:],
                    scale=temp,
                )

            # [feat | denom] = E^T @ [a | 1]; normalize; store.
            for mi in range(LT):
                po = psum_o.tile([128, H + 1], f32)
                for li in range(LT):
                    w = e[:, li, 128 * mi : 128 * (mi + 1)]
                    nc.tensor.matmul(
                        po[:, 0:512],
                        lhsT=w,
                        rhs=t2[:, li, 0:512],
                        start=(li == 0),
                        stop=(li == LT - 1),
                    )
                    mm2nd = nc.tensor.matmul(
                        po[:, 512 : H + 1],
                        lhsT=w,
                        rhs=t2[:, li, 512 : H + 1],
                        start=(li == 0),
                        stop=(li == LT - 1),
                    )
                    if REUSE_WEIGHTS:
                        mm2nd.ins.ldweights = False
                rc = rc_pool.tile([128, 1], f32)
                nc.vector.reciprocal(rc[:], po[:, H : H + 1])
                ot = out_pool.tile([128, H], f32)
                nc.scalar.activation(
                    out=ot[:],
                    in_=po[:, 0:H],
                    func=mybir.ActivationFunctionType.Copy,
                    scale=rc[:],
                )
                out_eng = nc.scalar if OUT_RING == "scalar" else nc.gpsimd
                out_eng.dma_start(out=o_v[:, mi, :], in_=ot[:])

    nc.compile()
    return nc


def _get_nc(temp: float, repeats: int = 1, bench: bool = False):
    key = (round(float(temp), 12), repeats, bench)
    if key not in _CACHE:
        _CACHE[key] = _build(float(temp), repeats, bench)
    return _CACHE[key]


def run(a, mask_a, temperature=None, trace=False):
    from concourse.bass_utils import run_bass_kernel_spmd

    a = np.ascontiguousarray(np.asarray(a, dtype=np.float32))
    mask_u8 = np.ascontiguousarray(np.asarray(mask_a)).view(np.uint8)
    if temperature is None:
        temperature = 1.0 / np.sqrt(np.float32(H))
    temp = float(np.asarray(temperature, dtype=np.float32))

    nc = _get_nc(temp)
    in_maps = [
        {
            "a": a[c * B_LOCAL : (c + 1) * B_LOCAL],
            "mask_a": mask_u8[c * B_LOCAL : (c + 1) * B_LOCAL],
        }
        for c in range(N_CORES)
    ]
    res = run_bass_kernel_spmd(
        nc, in_maps, core_ids=list(range(N_CORES)), trace=trace
    )
    out = np.concatenate([res.results[c]["out"] for c in range(N_CORES)], axis=0)
    return out, res


def kernel(a, mask_a, temperature=None, **_):
    out, _res = run(a, mask_a, temperature)
    return out

